# revision 1
# baseline (speedup 1.0000x reference)
"""Trainium2 Bass kernel for ConvGlobalLocalCapsuleLayer.

Per-capsule 3x3 SAME conv (8 capsules, 16->128 ch) + 3 iterations of dynamic
routing (softmax over output capsules, squash nonlinearity).

Sharding: data-parallel over batch B=32 across 8 cores (4 images/core),
weights replicated. No cross-core communication.

Per-core pipeline (positions chunked 512 = 16 h-rows):
  - load x natural [pos,128ch], PE-transpose, cast fp16 into x_sb
    [(i,cin), padded 34x34 geometry]
  - x3 im2col over dy only: [(i%2,dy,cin)=96, (i//2, img, 34, 34)] fp16;
    dx and dy tap shifts are free-dim AP offsets
  - conv: per (i,dx) K=96 fp16 matmul accumulating over dx; votes cast fp16
  - S = sum_i votes via 9 extra K=128 fp16 matmuls from x_sb (iter-1 route
    is uniform so only the capsule-sum is needed)
  - routing on-chip: DVE fp16 multiplies + fp32 squash chain;
    PE constant-matrix reductions/expansions (f-reduce via block-ones,
    i-reduce via identity, c-sum, f-replicate); ACT exp/sqrt/square/casts;
    reciprocal_approx_fast on DVE; logits accumulate in PSUM across iters
  - output transposed back via PE, DMA out [pos, 128]
"""
import sys

sys.path.insert(0, "/opt/trn_rl_repo")
sys.path.insert(0, "/root/.axon_site/_ro/trn_rl_repo")

import numpy as np
from concourse import bacc, mybir, tile
from concourse.bass_utils import run_bass_kernel_spmd

dt = mybir.dt
AF = mybir.ActivationFunctionType
OP = mybir.AluOpType

N_CORES = 8
EPS = 1e-7
IMG, HH, WW, ICAPS, CIN, C, F = 4, 32, 32, 8, 16, 8, 16
CF = 128
PH, PW = 34, 34
IMGLEN = PH * PW            # 1156
PADLEN = IMG * IMGLEN       # 4624
GUARD = 34
NCHUNK = 8                  # 512-position chunks (16 h-rows each)
TAPS = [(dy, dx) for dy in (-1, 0, 1) for dx in (-1, 0, 1)]

_CACHE = {}


def _host_constants(W, b):
    """All lhsT constant matrices + weight arrangements, built host-side."""
    W = np.asarray(W, np.float32)
    b = np.asarray(b, np.float32)
    cst = {}
    w_s = np.zeros((128, 9 * CF), np.float16)
    for t, (dy, dx) in enumerate(TAPS):
        w_s[:, t * CF:(t + 1) * CF] = W[:, dy + 1, dx + 1, :, :].reshape(128, CF)
    cst["w_s"] = w_s
    w_c = np.zeros((96, ICAPS * 3 * CF), np.float16)
    for i in range(ICAPS):
        q = i % 2
        for dxi in range(3):
            blk = np.zeros((96, CF), np.float16)
            for dyi in range(3):
                blk[q * 48 + dyi * 16:q * 48 + dyi * 16 + 16] = W[i, dyi, dxi]
            w_c[:, (i * 3 + dxi) * CF:(i * 3 + dxi + 1) * CF] = blk
    cst["w_c"] = w_c
    ei = np.zeros((CF, ICAPS * 64), np.float16)
    expi = np.zeros((64, ICAPS * CF), np.float16)
    for i in range(ICAPS):
        for c in range(C):
            for f in range(F):
                ei[c * F + f, i * 64 + i * C + c] = 1.0
                expi[i * C + c, i * CF + c * F + f] = 1.0
    cst["ei"] = ei
    cst["expi"] = expi
    onecc = np.zeros((CF, CF), np.float32)
    for c in range(C):
        onecc[c * F:(c + 1) * F, c * F:(c + 1) * F] = 1.0
    cst["onecc"] = onecc
    oneii = np.zeros((64, 64), np.float32)
    for i in range(ICAPS):
        oneii[i * C:(i + 1) * C, i * C:(i + 1) * C] = 1.0
    cst["oneii"] = oneii
    cst["idm16"] = np.eye(128, dtype=np.float16)
    cst["idm32"] = np.eye(128, dtype=np.float32)
    cst["bvec"] = b.reshape(CF, 1)
    cst["b8vec"] = 8.0 * b.reshape(CF, 1)
    cst["epsv"] = np.full((128, 1), EPS, np.float32)
    cst["lnkv"] = np.full((128, 1), np.log(0.125), np.float32)
    return cst


_CONST_SPECS = [
    ("w_s", [128, 9 * CF], dt.float16),
    ("w_c", [96, ICAPS * 3 * CF], dt.float16),
    ("ei", [CF, ICAPS * 64], dt.float16),
    ("expi", [64, ICAPS * CF], dt.float16),
    ("onecc", [CF, CF], dt.float32r),
    ("oneii", [64, 64], dt.float32r),
    ("idm16", [128, 128], dt.float16),
    ("idm32", [128, 128], dt.float32),
    ("bvec", [CF, 1], dt.float32),
    ("b8vec", [CF, 1], dt.float32),
    ("epsv", [128, 1], dt.float32),
    ("lnkv", [128, 1], dt.float32),
]


def _r(ap):
    """bitcast fp32 AP to float32r for full-rate PE streaming"""
    return ap.bitcast(dt.float32r)


def _build_program():
    nc = bacc.Bacc("TRN2", target_bir_lowering=False, debug=False)
    x_d = nc.dram_tensor("x", [IMG * HH * WW, 128], dt.float32,
                         kind="ExternalInput").ap()
    out_d = nc.dram_tensor("out", [IMG * HH * WW, 128], dt.float16,
                           kind="ExternalOutput").ap()
    cst_d = {n: nc.dram_tensor(n, sh, d, kind="ExternalInput").ap()
             for n, sh, d in _CONST_SPECS}

    with tile.TileContext(nc) as tc:
        with (
            tc.tile_pool(name="const", bufs=1) as cpool,
            tc.tile_pool(name="xbig", bufs=1) as xpool,
            tc.tile_pool(name="xnat", bufs=4) as npool,
            tc.tile_pool(name="work", bufs=2) as wpool,
            tc.tile_pool(name="med", bufs=2) as mpool,
            tc.tile_pool(name="ps_conv", bufs=2, space="PSUM") as ps_conv,
            tc.tile_pool(name="ps_pre", bufs=2, space="PSUM") as ps_pre,
            tc.tile_pool(name="ps_lz", bufs=2, space="PSUM") as ps_lz,
            tc.tile_pool(name="ps_tmp", bufs=2, space="PSUM") as ps_tmp,
        ):
            cst = {}
            for n, sh, d in _CONST_SPECS:
                t = cpool.tile(sh, d, tag=n)
                nc.sync.dma_start(t[:], cst_d[n][:])
                cst[n] = t

            # -------- x load + transpose + cast fp16, split in halves ------
            # two image-pair halves so chunk-0 conv starts after half the load
            HLEN = 2 * IMGLEN
            x_sbh = []
            x3h = []

            def xgeom2(ap):
                return ap.rearrange("p (im h w) -> p im h w", im=2, h=PH, w=PW)

            for half in range(2):
                xs = xpool.tile([128, 2 * GUARD + HLEN], dt.float16,
                                tag="x_sb", bufs=2)
                nc.gpsimd.memset(xs[:], 0.0)
                x_sbh.append(xs)
                for quad in range(4):
                    # one DMA covers 4 position sub-chunks: [128, 4, 128]
                    base = (half * 16 + quad * 4) * 128
                    xt = npool.tile([128, 4 * 128], dt.float32, tag="xnat")
                    src4 = x_d[base:base + 512, :].rearrange(
                        "(blk p) c -> p blk c", p=128)
                    nc.sync.dma_start(
                        xt[:].rearrange("p (blk c) -> p blk c", blk=4), src4)
                    for blk in range(4):
                        sub = quad * 4 + blk
                        img_loc, h0 = sub // 8, (sub % 8) * 4
                        tp = ps_conv.tile([128, 128], dt.float32, tag="conv")
                        nc.tensor.transpose(
                            tp[:], xt[:, blk * 128:(blk + 1) * 128],
                            cst["idm32"][:])
                        dst = xgeom2(xs[:, GUARD:GUARD + HLEN])[
                            :, img_loc, h0 + 1:h0 + 5, 1:33]
                        src = tp[:].rearrange("p (h w) -> p h w", h=4, w=WW)
                        nc.scalar.activation(dst, src, AF.Copy)
                x3 = xpool.tile([96, IMG * HLEN], dt.float16, tag="x3", bufs=2)
                for i in range(ICAPS):
                    q, g = i % 2, i // 2
                    for dyi, dy in enumerate((-1, 0, 1)):
                        src = xs[i * 16:(i + 1) * 16,
                                 GUARD + dy * PW:GUARD + dy * PW + HLEN]
                        dst = x3[q * 48 + dyi * 16:q * 48 + dyi * 16 + 16,
                                 g * HLEN:(g + 1) * HLEN]
                        nc.sync.dma_start(dst, src)
                x3h.append(x3)

            # ---------------- per-chunk conv + routing --------------------
            # Emitted as a 2-chunk routing wavefront with conv lookahead so
            # the (greedy, program-order) Tile scheduler can fill one chunk's
            # serial squash/softmax dependency chains with the sibling
            # chunk's PE/ACT/DVE work.
            st = [dict() for _ in range(NCHUNK)]

            def conv_S(ch):
                img, half = ch // 2, ch % 2
                h0 = 1 + 16 * half  # padded h of first output row

                half, img_loc = img // 2, img % 2

                def x3_rhs(i, dx):
                    g = i // 2
                    v = x3h[half][:, g * HLEN:(g + 1) * HLEN]
                    v = v.rearrange("p (im h w) -> p im h w", im=2, h=PH, w=PW)
                    return v[:, img_loc, h0:h0 + 16, 1 + dx:33 + dx]

                def xsb_rhs(dy, dx):
                    v = xgeom2(x_sbh[half][:, GUARD:GUARD + HLEN])
                    return v[:, img_loc, h0 + dy:h0 + dy + 16, 1 + dx:33 + dx]

                votes16 = wpool.tile([128, ICAPS * 512], dt.float16, tag="votes", bufs=3)
                v3 = votes16[:].rearrange("p (i n) -> p i n", i=ICAPS)
                for i in range(ICAPS):
                    vp = ps_conv.tile([128, 512], dt.float32, tag="conv")
                    vps = vp[:].rearrange("p (h w) -> p h w", h=16, w=WW)
                    for dxi, dx in enumerate((-1, 0, 1)):
                        lhsT = cst["w_c"][:, (i * 3 + dxi) * CF:(i * 3 + dxi + 1) * CF]
                        nc.tensor.matmul(vps, lhsT, x3_rhs(i, dx),
                                         start=(dxi == 0), stop=(dxi == 2))
                    nc.scalar.activation(v3[:, i, :], vp[:], AF.Copy)

                S = ps_pre.tile([128, 512], dt.float32, tag="pre")
                Ss = S[:].rearrange("p (h w) -> p h w", h=16, w=WW)
                for t, (dy, dx) in enumerate(TAPS):
                    nc.tensor.matmul(Ss, cst["w_s"][:, t * CF:(t + 1) * CF],
                                     xsb_rhs(dy, dx),
                                     start=(t == 0), stop=(t == 8))
                st[ch]["v3"] = v3
                st[ch]["votes16"] = votes16
                st[ch]["S"] = S

            def squash(pre_ps, b_ap, pre_scale, out_dtype, atag):
                """act = (pre_raw + b')*w, w = pre_scale*s2*r/(...)"""
                sq = mpool.tile([128, 512], dt.float32r, tag="sq")
                nc.scalar.activation(sq[:], pre_ps[:], AF.Square,
                                     bias=cst["bvec"][:], scale=pre_scale)
                s2 = ps_tmp.tile([128, 512], dt.float32, tag="tmp")
                nc.tensor.matmul(s2[:], cst["onecc"][:], sq[:],
                                 start=True, stop=True)
                sqt = mpool.tile([128, 512], dt.float32, tag="sqt")
                nc.scalar.activation(sqt[:], s2[:], AF.Sqrt,
                                     bias=cst["epsv"][:])
                u = mpool.tile([128, 512], dt.float32, tag="u")
                nc.vector.scalar_tensor_tensor(u[:], s2[:], 1.0, sqt[:],
                                               OP.add, OP.mult)
                r = mpool.tile([128, 512], dt.float32, tag="r")
                nc.vector.reciprocal_approx_fast(r[:], u[:])
                w = mpool.tile([128, 512], dt.float32, tag="w")
                nc.vector.scalar_tensor_tensor(w[:], s2[:], pre_scale, r[:],
                                               OP.mult, OP.mult)
                act = mpool.tile([128, 512], out_dtype, tag=atag)
                nc.vector.scalar_tensor_tensor(act[:], pre_ps[:], b_ap,
                                               w[:], OP.add, OP.mult)
                return act

            def iter1(ch):
                st[ch]["act16"] = squash(st[ch]["S"], cst["b8vec"][:], 0.125,
                                         dt.float16, "act1")
                lz = ps_lz.tile([64, 512], dt.float32, tag="lz")
                st[ch]["lz"] = lz

            def riter(ch, it):
                v3 = st[ch]["v3"]
                act16 = st[ch]["act16"]
                L = st[ch]["lz"][:]
                rv_a = wpool.tile([128, ICAPS * 512], dt.float16, tag="rv_a")
                ra3 = rv_a[:].rearrange("p (i n) -> p i n", i=ICAPS)
                act_b = act16[:].unsqueeze(1).broadcast_to([128, ICAPS, 512])
                nc.vector.tensor_tensor(ra3, v3, act_b, OP.mult)
                for i in range(ICAPS):
                    nc.tensor.matmul(L, cst["ei"][:, i * 64:(i + 1) * 64],
                                     ra3[:, i, :], start=(it == 2 and i == 0),
                                     stop=(it == 3 and i == ICAPS - 1),
                                     skip_group_check=True)
                ev = mpool.tile([64, 512], dt.float32r, tag="ev")
                nc.scalar.activation(ev[:], L, AF.Exp)
                Z = ps_tmp.tile([64, 512], dt.float32, tag="tmp")
                nc.tensor.matmul(Z[:], cst["oneii"][:], ev[:],
                                 start=True, stop=True)
                rz = mpool.tile([64, 512], dt.float32, tag="rz")
                nc.vector.reciprocal_approx_fast(rz[:], Z[:])
                route16 = mpool.tile([64, 512], dt.float16, tag="route16")
                nc.vector.tensor_tensor(route16[:], ev[:].bitcast(dt.float32),
                                        rz[:], OP.mult)

                rv_b = wpool.tile([128, ICAPS * 512], dt.float16, tag="rv_b")
                rb3 = rv_b[:].rearrange("p (i n) -> p i n", i=ICAPS)
                for i in range(ICAPS):
                    rr = ps_tmp.tile([128, 512], dt.float32, tag="tmp")
                    nc.tensor.matmul(rr[:],
                                     cst["expi"][:, i * CF:(i + 1) * CF],
                                     route16[:], start=True, stop=True)
                    nc.vector.tensor_tensor(rb3[:, i, :], v3[:, i, :],
                                            rr[:], OP.mult)
                pre = ps_pre.tile([128, 512], dt.float32, tag="pre")
                for i in range(ICAPS):
                    nc.tensor.matmul(pre[:], cst["idm16"][:], rb3[:, i, :],
                                     start=(i == 0), stop=(i == ICAPS - 1))
                if it == 2:
                    st[ch]["act16"] = squash(pre, cst["bvec"][:], 1.0,
                                             dt.float16, "act2")
                else:
                    st[ch]["act_f"] = squash(pre, cst["bvec"][:], 1.0,
                                             dt.float16, "actf")

            def out_chunk(ch):
                act_f = st[ch]["act_f"]
                tp = ps_conv.tile([128, 512], dt.float16, tag="conv")
                for si in range(4):
                    nc.tensor.transpose(tp[:, si * 128:(si + 1) * 128],
                                        act_f[:, si * 128:(si + 1) * 128],
                                        cst["idm16"][:])
                onat = mpool.tile([128, 512], dt.float16, tag="onat")
                nc.scalar.activation(onat[:], tp[:], AF.Copy)
                dst = out_d[ch * 512:(ch + 1) * 512, :].rearrange(
                    "(s p) c -> p s c", p=128)
                nc.sync.dma_start(dst, onat[:].rearrange(
                    "p (s c) -> p s c", s=4))
                st[ch].clear()

            conv_S(0)
            conv_S(1)
            iter1(0)
            iter1(1)
            for k in range(0, NCHUNK, 2):
                riter(k, 2)
                riter(k + 1, 2)
                if k + 2 < NCHUNK:
                    conv_S(k + 2)
                    iter1(k + 2)
                riter(k, 3)
                riter(k + 1, 3)
                if k + 3 < NCHUNK:
                    conv_S(k + 3)
                    iter1(k + 3)
                out_chunk(k)
                out_chunk(k + 1)

    nc.compile()
    return nc


def kernel(input_tensor, W, b):
    x = np.ascontiguousarray(np.asarray(input_tensor, np.float32))
    B = x.shape[0]
    per = B // N_CORES
    assert x.shape == (32, 32, 32, 8, 16) and per == IMG

    if "nc" not in _CACHE:
        _CACHE["nc"] = _build_program()
    nc = _CACHE["nc"]

    cst = _host_constants(W, b)
    in_maps = []
    for core in range(N_CORES):
        shard = x[core * per:(core + 1) * per].reshape(IMG * HH * WW, 128)
        m = {"x": np.ascontiguousarray(shard)}
        m.update(cst)
        in_maps.append(m)
    res = run_bass_kernel_spmd(nc, in_maps, list(range(N_CORES)))
    out = np.concatenate([res.results[c]["out"].reshape(IMG, HH, WW, C, F)
                          for c in range(N_CORES)], axis=0)
    return out.astype(np.float32)



# revision 26
# speedup vs baseline: 1.0877x; 1.0877x over previous
"""Trainium2 Bass kernel for ConvGlobalLocalCapsuleLayer.

Per-capsule 3x3 SAME conv (8 capsules, 16->128 ch) + 3 iterations of dynamic
routing (softmax over output capsules, squash nonlinearity).

Sharding: data-parallel over batch B=32 across 8 cores (4 images/core),
weights replicated. No cross-core communication.

Per-core pipeline (positions chunked 512 = 16 h-rows):
  - load x natural [pos,128ch], PE-transpose, cast fp16 into x_sb
    [(i,cin), padded 34x34 geometry]
  - x3 im2col over dy only: [(i%2,dy,cin)=96, (i//2, img, 34, 34)] fp16;
    dx and dy tap shifts are free-dim AP offsets
  - conv: per (i,dx) K=96 fp16 matmul accumulating over dx; votes cast fp16
  - S = sum_i votes via 9 extra K=128 fp16 matmuls from x_sb (iter-1 route
    is uniform so only the capsule-sum is needed)
  - routing on-chip: DVE fp16 multiplies + fp32 squash chain;
    PE constant-matrix reductions/expansions (f-reduce via block-ones,
    i-reduce via identity, c-sum, f-replicate); ACT exp/sqrt/square/casts;
    reciprocal_approx_fast on DVE; logits accumulate in PSUM across iters
  - output transposed back via PE, DMA out [pos, 128]
"""
import sys

sys.path.insert(0, "/opt/trn_rl_repo")
sys.path.insert(0, "/root/.axon_site/_ro/trn_rl_repo")

import numpy as np
from concourse import bacc, mybir, tile
from concourse.bass_utils import run_bass_kernel_spmd

dt = mybir.dt
AF = mybir.ActivationFunctionType
OP = mybir.AluOpType

N_CORES = 8
EPS = 1e-7
IMG, HH, WW, ICAPS, CIN, C, F = 4, 32, 32, 8, 16, 8, 16
CF = 128
PH, PW = 34, 34
IMGLEN = PH * PW            # 1156
PADLEN = IMG * IMGLEN       # 4624
GUARD = 34
NCHUNK = 8                  # 512-position chunks (16 h-rows each)
TAPS = [(dy, dx) for dy in (-1, 0, 1) for dx in (-1, 0, 1)]

_CACHE = {}


def _host_constants(W, b):
    """All lhsT constant matrices + weight arrangements, built host-side."""
    W = np.asarray(W, np.float32)
    b = np.asarray(b, np.float32)
    cst = {}
    w_s = np.zeros((128, 9 * CF), np.float16)
    for t, (dy, dx) in enumerate(TAPS):
        w_s[:, t * CF:(t + 1) * CF] = W[:, dy + 1, dx + 1, :, :].reshape(128, CF)
    cst["w_s"] = w_s
    w_c = np.zeros((96, ICAPS * 3 * CF), np.float16)
    for i in range(ICAPS):
        q = i % 2
        for dxi in range(3):
            blk = np.zeros((96, CF), np.float16)
            for dyi in range(3):
                blk[q * 48 + dyi * 16:q * 48 + dyi * 16 + 16] = W[i, dyi, dxi]
            w_c[:, (i * 3 + dxi) * CF:(i * 3 + dxi + 1) * CF] = blk
    cst["w_c"] = w_c
    ei = np.zeros((CF, ICAPS * 64), np.float16)
    expi = np.zeros((64, ICAPS * CF), np.float16)
    for i in range(ICAPS):
        for c in range(C):
            for f in range(F):
                ei[c * F + f, i * 64 + i * C + c] = 1.0
                expi[i * C + c, i * CF + c * F + f] = 1.0
    cst["ei"] = ei
    cst["expi"] = expi
    onecc = np.zeros((CF, CF), np.float32)
    for c in range(C):
        onecc[c * F:(c + 1) * F, c * F:(c + 1) * F] = 1.0
    cst["onecc"] = onecc
    oneii = np.zeros((64, 64), np.float32)
    for i in range(ICAPS):
        oneii[i * C:(i + 1) * C, i * C:(i + 1) * C] = 1.0
    cst["oneii"] = oneii
    cst["idm16"] = np.eye(128, dtype=np.float16)
    cst["idm32"] = np.eye(128, dtype=np.float32)
    cst["bvec"] = b.reshape(CF, 1)
    cst["b8vec"] = 8.0 * b.reshape(CF, 1)
    cst["epsv"] = np.full((128, 1), EPS, np.float32)
    cst["onev"] = np.full((128, 1), 1.0, np.float32)
    cst["lnkv"] = np.full((128, 1), np.log(0.125), np.float32)
    cst["onecc16"] = cst["onecc"].astype(np.float16)
    cst["oneii16"] = cst["oneii"].astype(np.float16)
    cst["oneii2"] = np.vstack([oneii, oneii])
    return cst


_CONST_SPECS = [
    ("w_s", [128, 9 * CF], dt.float16),
    ("w_c", [96, ICAPS * 3 * CF], dt.float16),
    ("ei", [CF, ICAPS * 64], dt.float16),
    ("expi", [64, ICAPS * CF], dt.float16),
    ("onecc", [CF, CF], dt.float32r),
    ("oneii", [64, 64], dt.float32r),
    ("idm16", [128, 128], dt.float16),
    ("idm32", [128, 128], dt.float32),
    ("bvec", [CF, 1], dt.float32),
    ("b8vec", [CF, 1], dt.float32),
    ("epsv", [128, 1], dt.float32),
    ("onev", [128, 1], dt.float32),
    ("lnkv", [128, 1], dt.float32),
    ("onecc16", [CF, CF], dt.float16),
    ("oneii16", [64, 64], dt.float16),
    ("oneii2", [128, 64], dt.float32r),
]


def _r(ap):
    """bitcast fp32 AP to float32r for full-rate PE streaming"""
    return ap.bitcast(dt.float32r)


def _build_program():
    nc = bacc.Bacc("TRN2", target_bir_lowering=False, debug=False)
    x_d = nc.dram_tensor("x", [IMG * HH * WW, 128], dt.float32,
                         kind="ExternalInput").ap()
    out_d = nc.dram_tensor("out", [IMG * HH * WW, 128], dt.float16,
                           kind="ExternalOutput").ap()
    cst_d = {n: nc.dram_tensor(n, sh, d, kind="ExternalInput").ap()
             for n, sh, d in _CONST_SPECS}

    with tile.TileContext(nc) as tc:
        with (
            tc.tile_pool(name="const", bufs=1) as cpool,
            tc.tile_pool(name="xbig", bufs=1) as xpool,
            tc.tile_pool(name="xnat", bufs=4) as npool,
            tc.tile_pool(name="work", bufs=2) as wpool,
            tc.tile_pool(name="med", bufs=2) as mpool,
            tc.tile_pool(name="ps_conv", bufs=2, space="PSUM") as ps_conv,
            tc.tile_pool(name="ps_pre", bufs=2, space="PSUM") as ps_pre,
            tc.tile_pool(name="ps_lz", bufs=2, space="PSUM") as ps_lz,
            tc.tile_pool(name="ps_tmp", bufs=2, space="PSUM") as ps_tmp,
        ):
            cst = {}
            for n, sh, d in _CONST_SPECS:
                t = cpool.tile(sh, d, tag=n)
                nc.sync.dma_start(t[:], cst_d[n][:])
                cst[n] = t

            # -------- x load + transpose + cast fp16, split in halves ------
            # two image-pair halves so chunk-0 conv starts after half the load
            HLEN = 2 * IMGLEN
            x_sbh = []
            x3h = []

            def xgeom2(ap):
                return ap.rearrange("p (im h w) -> p im h w", im=2, h=PH, w=PW)

            for half in range(2):
                xs = xpool.tile([128, 2 * GUARD + HLEN], dt.float16,
                                tag="x_sb", bufs=2)
                nc.gpsimd.memset(xs[:], 0.0)
                x_sbh.append(xs)
                for quad in range(4):
                    # one DMA covers 4 position sub-chunks: [128, 4, 128]
                    base = (half * 16 + quad * 4) * 128
                    xt = npool.tile([128, 4 * 128], dt.float32, tag="xnat")
                    src4 = x_d[base:base + 512, :].rearrange(
                        "(blk p) c -> p blk c", p=128)
                    nc.sync.dma_start(
                        xt[:].rearrange("p (blk c) -> p blk c", blk=4), src4)
                    for blk in range(4):
                        sub = quad * 4 + blk
                        img_loc, h0 = sub // 8, (sub % 8) * 4
                        tp = ps_conv.tile([128, 128], dt.float32, tag="conv")
                        nc.tensor.transpose(
                            tp[:], xt[:, blk * 128:(blk + 1) * 128],
                            cst["idm32"][:])
                        dst = xgeom2(xs[:, GUARD:GUARD + HLEN])[
                            :, img_loc, h0 + 1:h0 + 5, 1:33]
                        src = tp[:].rearrange("p (h w) -> p h w", h=4, w=WW)
                        nc.scalar.activation(dst, src, AF.Copy)
                x3 = xpool.tile([96, IMG * HLEN], dt.float16, tag="x3", bufs=2)
                for i in range(ICAPS):
                    q, g = i % 2, i // 2
                    for dyi, dy in enumerate((-1, 0, 1)):
                        src = xs[i * 16:(i + 1) * 16,
                                 GUARD + dy * PW:GUARD + dy * PW + HLEN]
                        dst = x3[q * 48 + dyi * 16:q * 48 + dyi * 16 + 16,
                                 g * HLEN:(g + 1) * HLEN]
                        nc.sync.dma_start(dst, src)
                x3h.append(x3)

            # ---------------- per-chunk conv + routing --------------------
            # Emitted as a 2-chunk routing wavefront with conv lookahead so
            # the (greedy, program-order) Tile scheduler can fill one chunk's
            # serial squash/softmax dependency chains with the sibling
            # chunk's PE/ACT/DVE work.
            st = [dict() for _ in range(NCHUNK)]

            def conv_S(ch):
                img, half = ch // 2, ch % 2
                h0 = 1 + 16 * half  # padded h of first output row

                half, img_loc = img // 2, img % 2

                def x3_rhs(i, dx):
                    g = i // 2
                    v = x3h[half][:, g * HLEN:(g + 1) * HLEN]
                    v = v.rearrange("p (im h w) -> p im h w", im=2, h=PH, w=PW)
                    return v[:, img_loc, h0:h0 + 16, 1 + dx:33 + dx]

                def xsb_rhs(dy, dx):
                    v = xgeom2(x_sbh[half][:, GUARD:GUARD + HLEN])
                    return v[:, img_loc, h0 + dy:h0 + dy + 16, 1 + dx:33 + dx]

                votes16 = wpool.tile([128, ICAPS * 512], dt.float16, tag="votes", bufs=3)
                v3 = votes16[:].rearrange("p (i n) -> p i n", i=ICAPS)
                for i in range(ICAPS):
                    vp = ps_conv.tile([128, 512], dt.float32, tag="conv")
                    vps = vp[:].rearrange("p (h w) -> p h w", h=16, w=WW)
                    for dxi, dx in enumerate((-1, 0, 1)):
                        lhsT = cst["w_c"][:, (i * 3 + dxi) * CF:(i * 3 + dxi + 1) * CF]
                        nc.tensor.matmul(vps, lhsT, x3_rhs(i, dx),
                                         start=(dxi == 0), stop=(dxi == 2))
                    nc.scalar.activation(v3[:, i, :], vp[:], AF.Copy)

                S = ps_pre.tile([128, 512], dt.float32, tag="pre")
                Ss = S[:].rearrange("p (h w) -> p h w", h=16, w=WW)
                for t, (dy, dx) in enumerate(TAPS):
                    nc.tensor.matmul(Ss, cst["w_s"][:, t * CF:(t + 1) * CF],
                                     xsb_rhs(dy, dx),
                                     start=(t == 0), stop=(t == 8))
                st[ch]["v3"] = v3
                st[ch]["votes16"] = votes16
                st[ch]["S"] = S

            def squash(pre_ps, b_ap, pre_scale, out_dtype, atag):
                """act = (pre_raw + b')*w with
                w = ps*sqrt(s2+eps)/(1+s2) = exp(0.5*ln(s2+eps) - ln(1+s2) + ln ps)
                (equals the reference scale ps*s2/((1+s2)*sqrt(s2+eps)) up to a
                (s2+eps)/s2 factor, i.e. 1e-7/s2 relative).  Ln/Exp/Square/Copy
                live in one ACT table set, so no mid-kernel table reloads."""
                sq = mpool.tile([128, 512], dt.float16, tag="sq")
                nc.scalar.activation(sq[:], pre_ps[:], AF.Square,
                                     bias=cst["bvec"][:], scale=pre_scale)
                s2 = ps_tmp.tile([128, 512], dt.float32, tag="tmp")
                nc.tensor.matmul(s2[:], cst["onecc16"][:], sq[:],
                                 start=True, stop=True)
                l1 = mpool.tile([128, 512], dt.float16, tag="l1")
                nc.scalar.activation(l1[:], s2[:], AF.Ln, bias=cst["epsv"][:])
                l2 = mpool.tile([128, 512], dt.float16, tag="l2")
                nc.scalar.activation(l2[:], s2[:], AF.Ln, bias=cst["onev"][:])
                # e = l1 - 2*l2; w = exp(0.5*e + ln ps) = ps*sqrt(s2+eps)/(1+s2)
                l2n = mpool.tile([128, 512], dt.float16, tag="l2n")
                nc.vector.tensor_scalar(l2n[:], l2[:], -2.0, None, OP.mult)
                e = mpool.tile([128, 512], dt.float16, tag="e")
                nc.vector.tensor_tensor(e[:], l1[:], l2n[:], OP.add)
                w = mpool.tile([128, 512], dt.float16, tag="w")
                lnps = cst["lnkv"][:] if pre_scale != 1.0 else 0.0
                nc.scalar.activation(w[:], e[:], AF.Exp, bias=lnps, scale=0.5)
                act = mpool.tile([128, 512], out_dtype, tag=atag)
                nc.vector.scalar_tensor_tensor(act[:], pre_ps[:], b_ap,
                                               w[:], OP.add, OP.mult)
                return act

            def iter1(ch):
                st[ch]["act16"] = squash(st[ch]["S"], cst["b8vec"][:], 0.125,
                                         dt.float16, "act1")
                lz = ps_lz.tile([64, 512], dt.float32, tag="lz")
                st[ch]["lz"] = lz
                st[ch]["p0"] = 0

            def riter(ch, it):
                v3 = st[ch]["v3"]
                act16 = st[ch]["act16"]
                L = st[ch]["lz"][:]
                rv_a = wpool.tile([128, ICAPS * 512], dt.float16, tag="rv_a")
                ra3 = rv_a[:].rearrange("p (i n) -> p i n", i=ICAPS)
                act_b = act16[:].unsqueeze(1).broadcast_to([128, ICAPS, 512])
                nc.vector.tensor_tensor(ra3, v3, act_b, OP.mult)
                for i in range(ICAPS):
                    nc.tensor.matmul(L, cst["ei"][:, i * 64:(i + 1) * 64],
                                     ra3[:, i, :], start=(it == 2 and i == 0),
                                     stop=(it == 3 and i == ICAPS - 1),
                                     skip_group_check=True)
                # softmax via route = exp(L - ln Z); avoids DVE reciprocal and
                # keeps every ACT func in the natural_log_exp table set
                ev = mpool.tile([64, 512], dt.float32r, tag="ev")
                nc.scalar.activation(ev[:], L, AF.Exp)
                Z = ps_tmp.tile([64, 512], dt.float32, tag="tmp")
                nc.tensor.matmul(Z[:], cst["oneii"][:], ev[:],
                                 start=True, stop=True)
                lz = mpool.tile([64, 512], dt.float32, tag="lz32")
                nc.scalar.activation(lz[:], Z[:], AF.Ln)
                dd = mpool.tile([64, 512], dt.float16, tag="dd")
                nc.vector.scalar_tensor_tensor(dd[:], L, 0.0, lz[:],
                                               OP.add, OP.subtract)
                route16 = mpool.tile([64, 512], dt.float16, tag="route16")
                nc.scalar.activation(route16[:], dd[:], AF.Exp)

                rv_b = wpool.tile([128, ICAPS * 512], dt.float16, tag="rv_b")
                rb3 = rv_b[:].rearrange("p (i n) -> p i n", i=ICAPS)
                for i in range(ICAPS):
                    rr = ps_tmp.tile([128, 512], dt.float32, tag="tmp")
                    nc.tensor.matmul(rr[:],
                                     cst["expi"][:, i * CF:(i + 1) * CF],
                                     route16[:], start=True, stop=True)
                    nc.vector.tensor_tensor(rb3[:, i, :], v3[:, i, :],
                                            rr[:], OP.mult)
                pre = ps_pre.tile([128, 512], dt.float32, tag="pre")
                for i in range(ICAPS):
                    nc.tensor.matmul(pre[:], cst["idm16"][:], rb3[:, i, :],
                                     start=(i == 0), stop=(i == ICAPS - 1))
                if it == 2:
                    st[ch]["act16"] = squash(pre, cst["bvec"][:], 1.0,
                                             dt.float16, "act2")
                else:
                    st[ch]["act_f"] = squash(pre, cst["bvec"][:], 1.0,
                                             dt.float16, "actf")

            def out_chunk(ch):
                act_f = st[ch]["act_f"]
                tp = ps_conv.tile([128, 512], dt.float16, tag="conv")
                for si in range(4):
                    nc.tensor.transpose(tp[:, si * 128:(si + 1) * 128],
                                        act_f[:, si * 128:(si + 1) * 128],
                                        cst["idm16"][:])
                onat = mpool.tile([128, 512], dt.float16, tag="onat")
                nc.scalar.activation(onat[:], tp[:], AF.Copy)
                dst = out_d[ch * 512:(ch + 1) * 512, :].rearrange(
                    "(s p) c -> p s c", p=128)
                nc.sync.dma_start(dst, onat[:].rearrange(
                    "p (s c) -> p s c", s=4))
                st[ch].clear()

            conv_S(0)
            conv_S(1)
            iter1(0)
            iter1(1)
            for k in range(0, NCHUNK, 2):
                riter(k, 2)
                riter(k + 1, 2)
                if k + 2 < NCHUNK:
                    conv_S(k + 2)
                    iter1(k + 2)
                riter(k, 3)
                riter(k + 1, 3)
                if k + 3 < NCHUNK:
                    conv_S(k + 3)
                    iter1(k + 3)
                out_chunk(k)
                out_chunk(k + 1)

    # All ACT funcs used (Copy, Square, Ln, Exp) live together in
    # act_func_set 6 ("natural_log_exp").  The default insertion pass picks
    # the first set containing each missed func (set 5 for Ln, set 0 for
    # Exp), which thrashes 1.3us table loads between them every chunk.
    # Place one load of set 6 at entry instead.
    def _single_table_load():
        blk = nc.main_func.blocks[0]
        inst = mybir.InstLoadActFuncSet(
            name=nc.get_next_instruction_name(), ins=[], outs=[],
            act_func_set_id=6)
        inst.engine = mybir.EngineType.Activation
        nc.register_instruction(inst)
        blk.instructions.insert(0, inst)

    nc.insert_act_table_loads = _single_table_load
    nc.compile()
    return nc


def kernel(input_tensor, W, b):
    x = np.ascontiguousarray(np.asarray(input_tensor, np.float32))
    B = x.shape[0]
    per = B // N_CORES
    assert x.shape == (32, 32, 32, 8, 16) and per == IMG

    if "nc" not in _CACHE:
        _CACHE["nc"] = _build_program()
    nc = _CACHE["nc"]

    cst = _host_constants(W, b)
    in_maps = []
    for core in range(N_CORES):
        shard = x[core * per:(core + 1) * per].reshape(IMG * HH * WW, 128)
        m = {"x": np.ascontiguousarray(shard)}
        m.update(cst)
        in_maps.append(m)
    res = run_bass_kernel_spmd(nc, in_maps, list(range(N_CORES)))
    out = np.concatenate([res.results[c]["out"].reshape(IMG, HH, WW, C, F)
                          for c in range(N_CORES)], axis=0)
    return out.astype(np.float32)

